# revision 54
# baseline (speedup 1.0000x reference)
"""GAT layer (DGL GATConv + BatchNorm + ELU + residual) on 8 Trainium2 cores.

Strategy (dst-sharded graph parallel):
  - Sort edges by destination; shard destination nodes across 8 cores
    (12544 slots/core = 98 blocks x 128 slots, load-balanced by degree).
  - Each core builds the full node table [feat | el] = [x@W | x@W@almat]
    (100353 rows x 136 f16; row 100352 is a sentinel with el=-60000) in its
    own HBM, then gathers 272B rows per edge with indirect DMA (the per-edge
    descriptor-generation stream on GpSimd is the critical path).
  - Per 128-edge tile: one-hot S (edge->slot) built on DVE via is_equal;
    er expanded edge-wise with a one-hot matmul; w = exp(lrelu(el+er) - 4)
    (the -4 bias cancels in the softmax and keeps fp16 w in range);
    messages scaled by w; scatter-reduce into PSUM via S^T @ [feat|w].
  - BatchNorm batch stats are global: launch 1 returns per-core partial
    sums, the host reduces 2x128 floats, launch 2 applies the affine fold
    a*h+c, ELU and the residual in channel-major layout.
"""
import sys
sys.path.insert(0, "/opt/trn_rl_repo")
import numpy as np

import concourse.bass as bass
import concourse.bacc as bacc
import concourse.mybir as mybir
import concourse.tile as tile
from concourse.bass_utils import run_bass_kernel_spmd

F32 = mybir.dt.float32
F16 = mybir.dt.float16
I32 = mybir.dt.int32

N = 100000
E = 1600000
IN_DIM = 128
H = 8
D = 16
HD = 128
NCORES = 8
NSHARD = 12500            # dst nodes per core (N / 8)
NBLK = 100                # blocks per core (12800 slots > 12500 nodes:
                          # slack lets most blocks stay under 16 tiles)
TPB = 17                  # max tiles per block
SLOTS = NBLK * 128        # 12800 slots per core
NTOT = NCORES * SLOTS     # 102400 padded node count
SENT = NTOT               # sentinel table row
ROW = IN_DIM + H          # 136 meaningful f16 slots per table row (feat | el)
RSTRIDE = 256             # f16 slots per table row (512B, dma_gather aligned)
SENT4 = NTOT // 4         # sentinel row id in the mod-4 interleaved view
NEG_SLOPE = 0.2
EPS = 1e-5
WBIAS = -4.0              # exp bias; cancels in softmax, keeps f16 w finite

LAST_EXEC_NS = [0, 0]

_cache = {}


def _build_launch1(caps):
    """caps: [NBLK, 4] tiles per (block, src%4 range); same for all cores."""
    caps = np.asarray(caps, np.int64)
    ntb = caps.sum(axis=1)          # tiles per block
    tiles_tot = int(ntb.sum())
    capmax = int(caps.max())
    toff = np.zeros(NBLK + 1, np.int64)
    np.cumsum(ntb, out=toff[1:])
    nc = bacc.Bacc("TRN2", target_bir_lowering=False, debug=False,
                   num_devices=NCORES)
    xTh = nc.dram_tensor("xTh", [128, NTOT], F16, kind="ExternalInput")
    xTp = nc.dram_tensor("xTp", [128, SLOTS], F32, kind="ExternalInput")
    Wfd = nc.dram_tensor("Wf", [128, ROW], F16, kind="ExternalInput")
    Wrd = nc.dram_tensor("Wr", [128, H], F32, kind="ExternalInput")
    iota_r = nc.dram_tensor("iota_r", [128, 128], F16, kind="ExternalInput")
    iota_c = nc.dram_tensor("iota_c", [128, 1], F32, kind="ExternalInput")
    I16 = mybir.dt.int16
    idxd = nc.dram_tensor("idx16", [128, tiles_tot * 8], I16, kind="ExternalInput")
    dslotd = nc.dram_tensor("dslot", [128, tiles_tot], F32, kind="ExternalInput")
    drowd = nc.dram_tensor("drow", [1, tiles_tot * 128], F16, kind="ExternalInput")

    h_out = nc.dram_tensor("h_out", [SLOTS, HD], F32, kind="ExternalOutput")
    st_out = nc.dram_tensor("st_out", [128, 2], F32, kind="ExternalOutput")
    table = nc.dram_tensor("table", [NTOT + 4, RSTRIDE], F16)

    NGA = NTOT // 1024  # 98 groups of 8 node tiles in phase A

    with tile.TileContext(nc) as tc:
        with (
            tc.tile_pool(name="const", bufs=1) as constp,
            tc.tile_pool(name="pa_sb", bufs=12) as pa_sb,
            tc.tile_pool(name="g4p", bufs=8) as g4p,
            tc.tile_pool(name="sp", bufs=8) as sp,
            tc.tile_pool(name="st4p", bufs=5) as st4p,
            tc.tile_pool(name="wp", bufs=6) as wp,
            tc.tile_pool(name="drp", bufs=3) as drp,
            tc.tile_pool(name="fin", bufs=3) as finp,
        ):
            # ---- constants ----
            iota_row = constp.tile([128, 128], F16)
            nc.sync.dma_start(out=iota_row[:], in_=iota_r[:])
            iota_col = constp.tile([128, 1], F32)
            nc.sync.dma_start(out=iota_col[:], in_=iota_c[:])
            ones_row = constp.tile([1, 128], F16)
            nc.vector.memset(ones_row[:], 1.0)
            ones_col16 = constp.tile([128, 1], F16)
            nc.vector.memset(ones_col16[:], 1.0)
            wbias_col = constp.tile([128, 1], F32)
            nc.vector.memset(wbias_col[:], WBIAS)

            Wf_sb = constp.tile([128, ROW], F16)
            nc.sync.dma_start(out=Wf_sb[:], in_=Wfd[:])
            Wr_sb = constp.tile([128, H], F32)
            nc.sync.dma_start(out=Wr_sb[:], in_=Wrd[:])

            # ---- sentinel rows (one per mod-4 range) ----
            sent_sb = constp.tile([4, ROW], F16)
            nc.vector.memset(sent_sb[:], 0.0)
            nc.vector.memset(sent_sb[:, IN_DIM:ROW], -60000.0)
            nc.sync.dma_start(out=table[NTOT:NTOT + 4, 0:ROW], in_=sent_sb[:])

            pa_scope = tc.tile_pool(name="pa_ps", bufs=6, space="PSUM")
            pa_ps = pa_scope.__enter__()

            # ---- phase A: full node table (groups of 8 tiles) ----
            for g in range(NGA):
                x8 = pa_sb.tile([128, 1024], F16, tag="x8")
                if g % 2 == 0:
                    nc.scalar.dma_start(out=x8[:], in_=xTh[:, g * 1024:(g + 1) * 1024])
                else:
                    nc.sync.dma_start(out=x8[:], in_=xTh[:, g * 1024:(g + 1) * 1024])
                row8 = pa_sb.tile([128, 8 * ROW], F16, tag="row8")
                for k in range(8):
                    ps = pa_ps.tile([128, ROW], F32, tag="pa")
                    nc.tensor.matmul(out=ps[:], lhsT=x8[:, k * 128:(k + 1) * 128],
                                     rhs=Wf_sb[:], start=True, stop=True)
                    if k % 2 == 0:
                        nc.vector.tensor_copy(out=row8[:, k * ROW:(k + 1) * ROW],
                                              in_=ps[:])
                    else:
                        nc.scalar.activation(row8[:, k * ROW:(k + 1) * ROW],
                                             ps[:],
                                             mybir.ActivationFunctionType.Copy)
                wq = nc.sync if g % 2 == 0 else nc.scalar
                wq.dma_start(
                    out=table[g * 1024:(g + 1) * 1024, 0:ROW].rearrange(
                        "(f p) c -> p f c", f=8),
                    in_=row8[:].rearrange("p (f c) -> p f c", c=ROW))

            # ---- er for own slots: f16 [128, 98*8] ----
            er_sb = constp.tile([128, NBLK * H], F16)
            for b in range(NBLK):
                xp_sb = pa_sb.tile([128, 128], F32, tag="xp")
                nc.scalar.dma_start(out=xp_sb[:], in_=xTp[:, b * 128:(b + 1) * 128])
                ps = pa_ps.tile([128, H], F32, tag="pa")
                nc.tensor.matmul(out=ps[:], lhsT=xp_sb[:],
                                 rhs=Wr_sb[:], start=True, stop=True)
                nc.vector.tensor_copy(out=er_sb[:, b * H:(b + 1) * H], in_=ps[:])

            pa_scope.__exit__(None, None, None)
            blk_scope = tc.tile_pool(name="blk_ps", bufs=1, space="PSUM")
            blk_ps = blk_scope.__enter__()
            erp_scope = tc.tile_pool(name="er_ps", bufs=2, space="PSUM")
            er_ps = erp_scope.__enter__()
            dt_scope = tc.tile_pool(name="dt_ps", bufs=2, space="PSUM")
            dt_ps = dt_scope.__enter__()
            st_scope = tc.tile_pool(name="stat_ps", bufs=1, space="PSUM")
            stat_ps = st_scope.__enter__()

            # ---- index preloads ----
            idx_sb = constp.tile([128, tiles_tot * 8], mybir.dt.int16)
            nc.sync.dma_start(out=idx_sb[:], in_=idxd[:])
            dslot_sb = constp.tile([128, tiles_tot], F32)
            nc.sync.dma_start(out=dslot_sb[:], in_=dslotd[:])
            tabv = table[:].rearrange("(i r) c -> i r c", r=4)

            # ---- stats accumulators (persist across blocks) ----
            s1_ps = stat_ps.tile([128, 1], F32)
            s2_ps = stat_ps.tile([128, 1], F32)

            # ---- phase B: paired blocks, one 1024-idx gather per (pair, r) ----
            capmax2 = max(int(caps[2 * p][r] + caps[2 * p + 1][r])
                          for p in range(NBLK // 2) for r in range(4))
            gcol = 0
            for P in range(NBLK // 2):
                b0, b1 = 2 * P, 2 * P + 1
                nt0 = int(caps[b0].sum())
                nt1 = int(caps[b1].sum())
                pair_start = gcol
                dr = drp.tile([1, 2 * TPB * 128], F16, tag="dr")
                nc.scalar.dma_start(
                    out=dr[:, :(nt0 + nt1) * 128],
                    in_=drowd[:, pair_start * 128:
                              (pair_start + nt0 + nt1) * 128])
                psbA = blk_ps.tile([128, ROW], F32, tag="blkA")
                psbB = blk_ps.tile([128, ROW], F32, tag="blkB")
                tcs = [0, 0]
                for r in range(4):
                    cap0 = int(caps[b0][r])
                    cap1 = int(caps[b1][r])
                    tot = cap0 + cap1
                    if tot == 0:
                        continue
                    gch = g4p.tile([128, capmax2 * RSTRIDE], F16, tag="g4")
                    if tot * 128 <= 1024:
                        nc.gpsimd.dma_gather(
                            out_ap=gch[:, :tot * RSTRIDE].rearrange(
                                "p (k c) -> p k c", c=RSTRIDE),
                            in_ap=tabv[:, r, :],
                            idxs_ap=idx_sb[:, gcol * 8:(gcol + tot) * 8],
                            num_idxs=tot * 128,
                            num_idxs_reg=tot * 128,
                            elem_size=RSTRIDE,
                            elem_step=4 * RSTRIDE)
                    else:
                        for (o, cp) in ((0, cap0), (cap0, cap1)):
                            if cp == 0:
                                continue
                            nc.gpsimd.dma_gather(
                                out_ap=gch[:, o * RSTRIDE:
                                           (o + cp) * RSTRIDE].rearrange(
                                    "p (k c) -> p k c", c=RSTRIDE),
                                in_ap=tabv[:, r, :],
                                idxs_ap=idx_sb[:, (gcol + o) * 8:
                                               (gcol + o + cp) * 8],
                                num_idxs=cp * 128,
                                num_idxs_reg=cp * 128,
                                elem_size=RSTRIDE,
                                elem_step=4 * RSTRIDE)
                    for which, (bb, off, cap, ntX) in enumerate(
                            ((b0, 0, cap0, nt0), (b1, cap0, cap1, nt1))):
                        if cap == 0:
                            continue
                        psb = psbA if which == 0 else psbB
                        tc = tcs[which]
                        for g in range((cap + 3) // 4):
                            t0 = g * 4
                            nt = min(4, cap - t0)
                            ne = nt * 128
                            pti = gcol - pair_start + off + t0
                            dtp = dt_ps.tile([128, 512], F32, tag="dt")
                            nc.tensor.matmul(
                                out=dtp[:, :ne], lhsT=ones_row[:],
                                rhs=dr[:, pti * 128:pti * 128 + ne],
                                start=True, stop=True)
                            st4 = st4p.tile([128, 512], F16, tag="st4")
                            nc.vector.tensor_scalar(
                                out=st4[:, :ne], in0=dtp[:, :ne],
                                scalar1=iota_col[:], scalar2=None,
                                op0=mybir.AluOpType.is_equal)
                            erp = er_ps.tile([128, 4 * H], F32, tag="erp")
                            for k in range(nt):
                                nc.tensor.matmul(
                                    out=erp[:, k * H:(k + 1) * H],
                                    lhsT=st4[:, k * 128:(k + 1) * 128],
                                    rhs=er_sb[:, bb * H:(bb + 1) * H],
                                    start=True, stop=True)
                            g4r = (gch[:, (off + t0) * RSTRIDE:
                                       (off + t0 + nt) * RSTRIDE]
                                   .rearrange("p (t c) -> p t c", c=RSTRIDE))
                            el_view = g4r[:, :, IN_DIM:ROW]
                            wsb = wp.tile([128, 4 * H], F16, tag="w")
                            w_v = wsb[:, :nt * H].rearrange(
                                "p (t h) -> p t h", h=H)
                            nc.vector.tensor_tensor(
                                out=w_v,
                                in0=erp[:, :nt * H].rearrange(
                                    "p (t h) -> p t h", h=H),
                                in1=el_view,
                                op=mybir.AluOpType.add)
                            w5 = wp.tile([128, 4 * H], F16, tag="w5")
                            nc.vector.tensor_scalar(
                                out=w5[:, :nt * H], in0=wsb[:, :nt * H],
                                scalar1=NEG_SLOPE, scalar2=None,
                                op0=mybir.AluOpType.mult)
                            nc.vector.tensor_tensor(
                                out=wsb[:, :nt * H], in0=wsb[:, :nt * H],
                                in1=w5[:, :nt * H],
                                op=mybir.AluOpType.max)
                            nc.scalar.activation(
                                el_view,
                                wsb[:, :nt * H].rearrange(
                                    "p (t h) -> p t h", h=H),
                                mybir.ActivationFunctionType.Exp,
                                bias=wbias_col[:])
                            feat_view = g4r[:, :, 0:IN_DIM]
                            w_b = (el_view
                                   .rearrange("p t (h one) -> p t h one",
                                              h=H, one=1)
                                   .to_broadcast([128, nt, H, D]))
                            nc.vector.tensor_tensor(
                                out=feat_view.rearrange(
                                    "p t (h d) -> p t h d", d=D),
                                in0=feat_view.rearrange(
                                    "p t (h d) -> p t h d", d=D),
                                in1=w_b,
                                op=mybir.AluOpType.mult)
                            for k in range(nt):
                                col = gcol + off + t0 + k
                                s_sb = sp.tile([128, 128], F16, tag="s")
                                nc.vector.tensor_scalar(
                                    out=s_sb[:], in0=iota_row[:],
                                    scalar1=dslot_sb[:, col:col + 1],
                                    scalar2=None,
                                    op0=mybir.AluOpType.is_equal)
                                tk = tc + t0 + k
                                nc.tensor.matmul(
                                    out=psb[:],
                                    lhsT=s_sb[:],
                                    rhs=gch[:, (off + t0 + k) * RSTRIDE:
                                            (off + t0 + k) * RSTRIDE + ROW],
                                    start=(tk == 0), stop=(tk == ntX - 1))
                        tcs[which] = tc + cap
                    gcol += tot
                # ---- pair finalize (both blocks) ----
                for bb, psb in ((b0, psbA), (b1, psbB)):
                    ssum = finp.tile([128, H], F32, tag="ssum")
                    nc.vector.tensor_scalar(
                        out=ssum[:], in0=psb[:, IN_DIM:ROW],
                        scalar1=1e-30, scalar2=None,
                        op0=mybir.AluOpType.add)
                    rec = finp.tile([128, H], F32, tag="rec")
                    nc.vector.reciprocal(out=rec[:], in_=ssum[:])
                    h_sb = finp.tile([128, HD], F32, tag="h")
                    rec_b = (rec[:].rearrange("p (h one) -> p h one",
                                              h=H, one=1)
                             .to_broadcast([128, H, D]))
                    nc.vector.tensor_tensor(
                        out=h_sb[:].rearrange("p (h d) -> p h d", d=D),
                        in0=psb[:, 0:IN_DIM].rearrange(
                            "p (h d) -> p h d", d=D),
                        in1=rec_b, op=mybir.AluOpType.mult)
                    h16 = finp.tile([128, HD], F16, tag="h16")
                    nc.vector.tensor_copy(out=h16[:], in_=h_sb[:])
                    sq_sb = finp.tile([128, HD], F16, tag="sq")
                    nc.scalar.activation(sq_sb[:], h_sb[:],
                                         mybir.ActivationFunctionType.Square)
                    nc.tensor.matmul(out=s1_ps[:], lhsT=h16[:],
                                     rhs=ones_col16[:],
                                     start=(bb == 0), stop=(bb == NBLK - 1))
                    nc.tensor.matmul(out=s2_ps[:], lhsT=sq_sb[:],
                                     rhs=ones_col16[:],
                                     start=(bb == 0), stop=(bb == NBLK - 1))
                    nc.sync.dma_start(out=h_out[bb * 128:(bb + 1) * 128, :],
                                      in_=h_sb[:])

            stat_sb = constp.tile([128, 2], F32)
            nc.vector.tensor_copy(out=stat_sb[:, 0:1], in_=s1_ps[:])
            nc.vector.tensor_copy(out=stat_sb[:, 1:2], in_=s2_ps[:])
            nc.sync.dma_start(out=st_out[:], in_=stat_sb[:])
            st_scope.__exit__(None, None, None)
            dt_scope.__exit__(None, None, None)
            erp_scope.__exit__(None, None, None)
            blk_scope.__exit__(None, None, None)

    nc.compile()
    return nc


def _build_launch2():
    nc = bacc.Bacc("TRN2", target_bir_lowering=False, debug=False,
                   num_devices=NCORES)
    h_in = nc.dram_tensor("h_in", [SLOTS, HD], F32, kind="ExternalInput")
    xTp = nc.dram_tensor("xTp", [128, SLOTS], F32, kind="ExternalInput")
    ac = nc.dram_tensor("ac", [128, 2], F32, kind="ExternalInput")
    out_t = nc.dram_tensor("out_t", [128, SLOTS], F32, kind="ExternalOutput")

    CH = 512
    NCH = SLOTS // CH  # 24 full + 1 tail of 256
    chunks = [(i * CH, CH) for i in range(NCH)]
    if SLOTS % CH:
        chunks.append((NCH * CH, SLOTS % CH))

    with tile.TileContext(nc) as tc:
        with (
            tc.tile_pool(name="const", bufs=1) as constp,
            tc.tile_pool(name="ld", bufs=4) as ldp,
            tc.tile_pool(name="ps", bufs=3, space="PSUM") as psp,
            tc.tile_pool(name="wk", bufs=3) as wkp,
        ):
            from concourse.masks import make_identity
            ident = constp.tile([128, 128], F32)
            make_identity(nc, ident[:])
            ac_sb = constp.tile([128, 2], F32)
            nc.sync.dma_start(out=ac_sb[:], in_=ac[:])

            for (o, w) in chunks:
                nk = w // 128
                hp = psp.tile([128, CH], F32, tag="hp")
                for k in range(nk):
                    hl = ldp.tile([128, 128], F32, tag="hl")
                    nc.sync.dma_start(
                        out=hl[:], in_=h_in[o + k * 128:o + (k + 1) * 128, :])
                    nc.tensor.transpose(out=hp[:, k * 128:(k + 1) * 128],
                                        in_=hl[:], identity=ident[:])
                h2 = wkp.tile([128, CH], F32, tag="h2")
                nc.vector.tensor_scalar(out=h2[:, :w], in0=hp[:, :w],
                                        scalar1=ac_sb[:, 0:1],
                                        scalar2=ac_sb[:, 1:2],
                                        op0=mybir.AluOpType.mult,
                                        op1=mybir.AluOpType.add)
                m = wkp.tile([128, CH], F32, tag="m")
                nc.vector.tensor_scalar(out=m[:, :w], in0=h2[:, :w],
                                        scalar1=0.0, scalar2=None,
                                        op0=mybir.AluOpType.min)
                nc.scalar.activation(m[:, :w], m[:, :w],
                                     mybir.ActivationFunctionType.Exp)
                nc.vector.tensor_scalar(out=m[:, :w], in0=m[:, :w],
                                        scalar1=-1.0, scalar2=None,
                                        op0=mybir.AluOpType.add)
                # elu = max(h2, exp(min(h2,0))-1)
                nc.vector.tensor_tensor(out=h2[:, :w], in0=h2[:, :w],
                                        in1=m[:, :w],
                                        op=mybir.AluOpType.max)
                xt = ldp.tile([128, CH], F32, tag="xt")
                nc.sync.dma_start(out=xt[:, :w], in_=xTp[:, o:o + w])
                nc.vector.tensor_tensor(out=h2[:, :w], in0=h2[:, :w],
                                        in1=xt[:, :w], op=mybir.AluOpType.add)
                nc.sync.dma_start(out=out_t[:, o:o + w], in_=h2[:, :w])

    nc.compile()
    return nc


def _host_prep(x, src, dst):
    """Shard + balance + pad. Returns per-core index data and shared ntb."""
    raw = []
    cnts = np.zeros((NCORES, NBLK, 4), np.int64)
    RCAP = 512  # per-(block, range) soft edge cap = 4 tiles
    for c in range(NCORES):
        lo = c * NSHARD
        hi = min((c + 1) * NSHARD, N)
        nodes_c = hi - lo
        m = (dst >= lo) & (dst < hi)
        e_src = src[m].astype(np.int64)
        e_dstl = (dst[m] - lo).astype(np.int64)
        e_rng = (e_src % 4).astype(np.int64)
        # per-node 4-vector of range in-degrees
        rdeg = np.zeros((nodes_c, 4), np.int64)
        np.add.at(rdeg, (e_dstl, e_rng), 1)
        deg = rdeg.sum(axis=1)
        order = np.argsort(-deg, kind="stable")
        # greedy vector packing: keep every (block, range) load <= RCAP
        loads = np.zeros((NBLK, 4), np.int64)
        slots_used = np.zeros(NBLK, np.int64)
        blk_of = np.empty(nodes_c, np.int64)
        slot_of = np.empty(nodes_c, np.int64)
        for v in order:
            rv = rdeg[v]
            open_b = slots_used < 128
            cand = loads + rv[None, :]
            feas = open_b & (cand <= RCAP).all(axis=1)
            if feas.any():
                # minimize the resulting worst per-range load (then total)
                score = cand.max(axis=1) * 4096 + cand.sum(axis=1)
                b = int(np.argmin(np.where(feas, score, 1 << 60)))
            else:
                over = np.maximum(cand - RCAP, 0).sum(axis=1)
                over_m = np.where(open_b, over, 1 << 60)
                b = int(np.argmin(over_m))
            blk_of[v] = b
            slot_of[v] = slots_used[b]
            slots_used[b] += 1
            loads[b] += rv
        assert loads.max() <= TPB * 128
        eb = blk_of[e_dstl]
        key = eb * 4 + e_rng
        cnt = np.bincount(key, minlength=NBLK * 4).reshape(NBLK, 4)
        eorder = np.argsort(key, kind="stable")
        offs = np.zeros(NBLK * 4 + 1, np.int64)
        np.cumsum(cnt.reshape(-1), out=offs[1:])
        # node index per slot (-1 for pad slots)
        node_of_slot = np.full(SLOTS, -1, np.int64)
        node_of_slot[blk_of * 128 + slot_of] = np.arange(nodes_c) + lo
        src4_sorted = (e_src[eorder] // 4).astype(np.int16)
        slot_sorted = slot_of[e_dstl[eorder]].astype(np.float32)
        raw.append((src4_sorted, slot_sorted, offs, node_of_slot))
        cnts[c] = cnt
    caps = (cnts.max(axis=0) + 127) // 128  # [NBLK, 4] tiles per chunk
    tiles_tot = int(caps.sum())
    per_core = []
    for c in range(NCORES):
        src4_sorted, slot_sorted, offs, node_of_slot = raw[c]
        idx_arr = np.full((128, tiles_tot * 8), SENT4, np.int16)
        dslot_arr = np.full((128, tiles_tot), 300.0, np.float32)
        drow_arr = np.full((1, tiles_tot * 128), 300.0, np.float16)
        t0 = 0
        for P in range(NBLK // 2):
            for r in range(4):
              for b in (2 * P, 2 * P + 1):
                nb = int(caps[b][r])
                if nb == 0:
                    continue
                k = offs[b * 4 + r + 1] - offs[b * 4 + r]
                seg_src = np.full(nb * 128, SENT4, np.int16)
                seg_slot = np.full(nb * 128, 300.0, np.float32)
                seg_src[:k] = src4_sorted[offs[b * 4 + r]:offs[b * 4 + r + 1]]
                seg_slot[:k] = slot_sorted[offs[b * 4 + r]:offs[b * 4 + r + 1]]
                # dma_gather snake: idx i -> partition i%16, col i//16, x8 rep
                idx_arr[:, t0 * 8:(t0 + nb) * 8] = np.tile(
                    seg_src.reshape(nb * 8, 16).T, (8, 1))
                dslot_arr[:, t0:t0 + nb] = seg_slot.reshape(nb, 128).T
                drow_arr[0, t0 * 128:(t0 + nb) * 128] = \
                    seg_slot.astype(np.float16)
                t0 += nb
        per_core.append((idx_arr, dslot_arr, drow_arr, node_of_slot))
    return per_core, caps


def kernel(x, src, dst, W, attn_l, attn_r, bias, gamma, beta):
    global LAST_EXEC_NS
    x = np.asarray(x, np.float32)
    src = np.asarray(src, np.int32)
    dst = np.asarray(dst, np.int32)
    W = np.asarray(W, np.float32)
    attn_l = np.asarray(attn_l, np.float32)
    attn_r = np.asarray(attn_r, np.float32)
    gamma = np.asarray(gamma, np.float32)
    beta = np.asarray(beta, np.float32)

    per_core, caps = _host_prep(x, src, dst)

    key = ("l1", caps.tobytes())
    if key not in _cache:
        _cache[key] = _build_launch1(caps)
    if "l2" not in _cache:
        _cache["l2"] = _build_launch2()
    nc1, nc2 = _cache[key], _cache["l2"]

    xT_h = np.zeros((128, NTOT), np.float16)
    xT_h[:, :N] = x.T.astype(np.float16)
    almat = np.zeros((HD, H), np.float32)
    armat = np.zeros((HD, H), np.float32)
    for h in range(H):
        almat[h * D:(h + 1) * D, h] = attn_l[h]
        armat[h * D:(h + 1) * D, h] = attn_r[h]
    Wf = np.concatenate([W, W @ almat], axis=1).astype(np.float16)  # [128,136]
    Wr = (W @ armat).astype(np.float32)                             # [128,8]
    iota_r = np.tile(np.arange(128, dtype=np.float16), (128, 1))
    iota_c = np.arange(128, dtype=np.float32).reshape(128, 1)

    in_maps = []
    xTp_list = []
    for c in range(NCORES):
        idx_arr, dslot_arr, drow_arr, node_of_slot = per_core[c]
        xTp = np.zeros((128, SLOTS), np.float32)
        real = node_of_slot >= 0
        xTp[:, real] = x[node_of_slot[real]].T
        xTp_list.append(xTp)
        in_maps.append({
            "xTh": xT_h, "xTp": xTp, "Wf": Wf, "Wr": Wr,
            "iota_r": iota_r, "iota_c": iota_c,
            "idx16": idx_arr, "dslot": dslot_arr, "drow": drow_arr,
        })

    res1 = run_bass_kernel_spmd(nc1, in_maps, list(range(NCORES)),
                                **_trace_kwargs())
    LAST_EXEC_NS[0] = res1.exec_time_ns or 0

    # host: combine BN stats (2x128 floats per core)
    S1 = np.zeros(128, np.float64)
    S2 = np.zeros(128, np.float64)
    for c in range(NCORES):
        st = res1.results[c]["st_out"]
        S1 += st[:, 0]
        S2 += st[:, 1]
    mu = (S1 / N).astype(np.float32)
    var = (S2 / N - (S1 / N) ** 2).astype(np.float32)
    a = gamma / np.sqrt(var + EPS)
    cc = beta - a * mu
    ac = np.stack([a, cc], axis=1).astype(np.float32)

    in_maps2 = []
    for c in range(NCORES):
        in_maps2.append({
            "h_in": res1.results[c]["h_out"],
            "xTp": xTp_list[c],
            "ac": ac,
        })
    res2 = run_bass_kernel_spmd(nc2, in_maps2, list(range(NCORES)),
                                **_trace_kwargs())
    LAST_EXEC_NS[1] = res2.exec_time_ns or 0

    out = np.zeros((N, IN_DIM), np.float32)
    for c in range(NCORES):
        node_of_slot = per_core[c][3]
        real = node_of_slot >= 0
        ot = res2.results[c]["out_t"]  # [128, SLOTS]
        out[node_of_slot[real]] = ot[:, real].T
    return out


def _trace_kwargs():
    import os
    if os.environ.get("GAT_TRACE", "0") == "1":
        return {"trace": True}
    return {}
